# revision 1
# baseline (speedup 1.0000x reference)
"""Multi-head attention Trainium2 kernel.

B=8, S=1024, D=1024, H=16 heads, head_dim=64.
Sharding: pure data parallelism over batch — one batch element per
NeuronCore, weights replicated, no collectives.

Per-core dataflow (all matmul operands bf16, fp32 PSUM accumulate):
  host:   xT = x.T (k-major) for q/k/v, WT = W.T for all weights, bf16.
  QT[do,s] = sum_k WqT[k,do]*xqT[k,s]   (+bq via ACT per-partition bias)
  KT[do,s] likewise
  V[s,dv]  = sum_k xvT[k,s]*WvT[k,dv]   (+bv via K=1 ones-row matmul)
             scattered into V65 layout: per head pair [V_e(64)|1|1|V_o(64)]
  per (head-pair, i-chunk):
    ST[j,i] = KT_h[d,j].T @ QT_h[d,i]   (K=64, two heads row-tiled)
    expST   = exp(ST/8)                  (ACT, PSUM->SBUF bf16)
    AV: psum[j->] = V65_slice.T @ expST  -> rows: OT_h + colsum row (ones col)
    normalize: DVE recip + gpsimd partition_broadcast + DVE mul -> OT[dv,i] bf16
  out[s,do] = sum_dv OT[dv,s]*WoT[dv,do] (+bo via ones-row) -> fp32 -> DRAM
"""

import numpy as np
import ml_dtypes
from contextlib import ExitStack

import concourse.bass as bass
import concourse.tile as tile
import concourse.mybir as mybir
from concourse import bacc
from concourse.bass_utils import run_bass_kernel_spmd

BF16 = mybir.dt.bfloat16
F32 = mybir.dt.float32
F32R = mybir.dt.float32r
AF = mybir.ActivationFunctionType

S = 1024
D = 1024
H = 16
HD = 64
P = 128
KT = D // P      # 8 contraction tiles
MT = S // P      # 8 row tiles
NC = 512         # free-dim chunk (one PSUM bank of fp32)
NCH = S // NC    # 2 chunks
PAIRS = H // 2   # 8
VW = 160  # per-pair V65 width [V_e(64) | ones@64 | gap 65-95 | V_o@96-159]
N_CORES = 8


def build_body(ctx: ExitStack, tc, io, dbg=None):
    nc = tc.nc

    const = ctx.enter_context(tc.tile_pool(name="const", bufs=1))
    qkt = ctx.enter_context(tc.tile_pool(name="qkt", bufs=1))
    v65p = ctx.enter_context(tc.tile_pool(name="v65", bufs=1))
    otp = ctx.enter_context(tc.tile_pool(name="otp", bufs=1))
    xw = ctx.enter_context(tc.tile_pool(name="xw", bufs=1))
    wop = ctx.enter_context(tc.tile_pool(name="wop", bufs=1))
    expp = ctx.enter_context(tc.tile_pool(name="expp", bufs=2))
    sb = ctx.enter_context(tc.tile_pool(name="sb", bufs=2))
    osbp = ctx.enter_context(tc.tile_pool(name="osb", bufs=3))
    psA = ctx.enter_context(tc.tile_pool(name="psA", bufs=1, space="PSUM"))
    psB = ctx.enter_context(tc.tile_pool(name="psB", bufs=4, space="PSUM"))

    def dump_sbuf(key, tile_ap, row0=0):
        if dbg is not None and key in dbg:
            nc.gpsimd.dma_start(
                dbg[key][row0 : row0 + tile_ap.shape[0], :], tile_ap)

    def dump_psum(key, ps_ap):
        if dbg is not None and key in dbg:
            t = osbp.tile(list(ps_ap.shape), F32, tag="dbgcp", name="dbgcp")
            nc.vector.tensor_copy(t[:], ps_ap)
            nc.sync.dma_start(dbg[key][:, :], t[:])

    # ---- constants ----
    ones = const.tile([1, P], BF16, tag="ones")
    nc.vector.memset(ones[:], 1.0)
    ones128 = const.tile([P, P], BF16, tag="ones128")
    nc.vector.memset(ones128[:], 1.0)
    bqc = const.tile([P, KT], F32, tag="bqc")
    nc.sync.dma_start(bqc[:], io["bq_c"][:])
    bkc = const.tile([P, KT], F32, tag="bkc")
    nc.sync.dma_start(bkc[:], io["bk_c"][:])
    bvr = const.tile([1, D], BF16, tag="bvr")
    nc.sync.dma_start(bvr[:], io["bv_r"][:])
    bor = const.tile([1, D], BF16, tag="bor")
    nc.sync.dma_start(bor[:], io["bo_r"][:])

    # ---- persistent activation tiles ----
    QT = [qkt.tile([P, S], BF16, tag=f"qt{m}", name=f"qt{m}") for m in range(MT)]
    KTt = [qkt.tile([P, S], BF16, tag=f"kt{m}", name=f"ktt{m}") for m in range(MT)]
    V65 = [v65p.tile([P, PAIRS * VW], BF16, tag=f"v65_{m}", name=f"v65_{m}") for m in range(MT)]
    OT = [otp.tile([P, S], BF16, tag=f"ot{m}", name=f"ot{m}") for m in range(MT)]

    # ones column at col 64 of each 160-wide pair block; zero the gap
    for m in range(MT):
        v = V65[m].rearrange("p (pr w) -> p pr w", w=VW)
        nc.vector.memset(v[:, :, HD : HD + 1], 1.0)
        nc.vector.memset(v[:, :, HD + 1 : 96], 0.0)

    def load_xw(xdram, wdram):
        xt = [xw.tile([P, S], BF16, tag=f"x{k}", name=f"xt{k}") for k in range(KT)]
        wt = [xw.tile([P, D], BF16, tag=f"w{k}", name=f"wt{k}") for k in range(KT)]
        for k in range(KT):
            nc.sync.dma_start(xt[k][:], xdram[k * P : (k + 1) * P, :])
            nc.sync.dma_start(wt[k][:], wdram[k * P : (k + 1) * P, :])
        return xt, wt

    # ---------- V projection (first: V65 needed by all attention) ----------
    xt, wt = load_xw(io["xvT"], io["wvT"])
    for m in range(MT):          # s tile
        for c in range(NCH):     # dv chunk
            ps = psB.tile([P, NC], F32, tag="ps", name="ps")
            for k in range(KT):
                nc.tensor.matmul(
                    ps[:],
                    xt[k][:, m * P : (m + 1) * P],
                    wt[k][:, c * NC : (c + 1) * NC],
                    start=(k == 0),
                    stop=False,
                )
            nc.tensor.matmul(
                ps[:], ones[0:1, :], bvr[0:1, c * NC : (c + 1) * NC],
                start=False, stop=True,
            )
            # scatter heads into V65 pair layout
            psv = ps.rearrange("p (pr two x) -> p pr two x", two=2, x=HD)
            v = V65[m].rearrange("p (pr w) -> p pr w", w=VW)
            pr0 = c * (NC // (2 * HD))  # first pair index in this chunk
            npr = NC // (2 * HD)        # pairs per chunk (4)
            nc.vector.tensor_copy(v[:, pr0 : pr0 + npr, 0:HD], psv[:, :, 0, :])
            nc.vector.tensor_copy(v[:, pr0 : pr0 + npr, 96:VW], psv[:, :, 1, :])

    # ---------- K projection ----------
    xt, wt = load_xw(io["xkT"], io["wkT"])
    for m in range(MT):          # do tile
        for c in range(NCH):     # s chunk
            ps = psB.tile([P, NC], F32, tag="ps", name="ps")
            for k in range(KT):
                nc.tensor.matmul(
                    ps[:],
                    wt[k][:, m * P : (m + 1) * P],
                    xt[k][:, c * NC : (c + 1) * NC],
                    start=(k == 0),
                    stop=(k == KT - 1),
                )
            nc.vector.tensor_scalar_add(
                KTt[m][:, c * NC : (c + 1) * NC], ps[:], bkc[:, m : m + 1])

    # ---------- Q projection ----------
    xt, wt = load_xw(io["xqT"], io["wqT"])
    for m in range(MT):
        for c in range(NCH):
            ps = psB.tile([P, NC], F32, tag="ps", name="ps")
            for k in range(KT):
                nc.tensor.matmul(
                    ps[:],
                    wt[k][:, m * P : (m + 1) * P],
                    xt[k][:, c * NC : (c + 1) * NC],
                    start=(k == 0),
                    stop=(k == KT - 1),
                )
            nc.vector.tensor_scalar_add(
                QT[m][:, c * NC : (c + 1) * NC], ps[:], bqc[:, m : m + 1])

    if dbg is not None:
        for m in range(MT):
            dump_sbuf("qt", QT[m][:], m * P)
            dump_sbuf("kt", KTt[m][:], m * P)
            dump_sbuf("v65", V65[m][:], m * P)

    # WoT loads overlap attention (xw w-slots free after Q projection)
    wo_t = [wop.tile([P, D], BF16, tag=f"wo{k}", name=f"wo{k}") for k in range(KT)]
    for k in range(KT):
        nc.sync.dma_start(wo_t[k][:], io["woT"][k * P : (k + 1) * P, :])

    # ---------- attention ----------
    def emit_scores(p, c):
        """ST[j,i] for heads 2p (partitions 0-63) and 2p+1 (64-127)."""
        expE = expp.tile([P, KT * NC], BF16, tag="expE")
        expO = expp.tile([P, KT * NC], BF16, tag="expO")
        for jj in range(4):                       # j-tile pairs
            sE = psA.tile([P, 2 * NC], F32, tag="sE", name="sE")
            sO = psA.tile([P, 2 * NC], F32, tag="sO", name="sO")
            for dj in range(2):
                j = 2 * jj + dj
                nc.tensor.matmul(
                    sE[:, dj * NC : (dj + 1) * NC],
                    KTt[p][0:HD, j * P : (j + 1) * P],
                    QT[p][0:HD, c * NC : (c + 1) * NC],
                    start=True, stop=True,
                )
                nc.tensor.matmul(
                    sO[:, dj * NC : (dj + 1) * NC],
                    KTt[p][HD:P, j * P : (j + 1) * P],
                    QT[p][HD:P, c * NC : (c + 1) * NC],
                    start=True, stop=True,
                )
            nc.scalar.activation(
                expE[:, 2 * jj * NC : (2 * jj + 2) * NC], sE[:], AF.Exp,
                scale=0.125)
            nc.scalar.activation(
                expO[:, 2 * jj * NC : (2 * jj + 2) * NC], sO[:], AF.Exp,
                scale=0.125)
        if p == 0 and c == 0:
            dump_sbuf("expE", expE[:])
            dump_sbuf("expO", expO[:])
        return expE, expO

    def emit_av(p, c, expE, expO):
        avE = psB.tile([P, NC], F32, tag="ps", name="avE")
        avO = psB.tile([P, NC], F32, tag="ps", name="avO")
        for jt in range(KT):
            nc.tensor.matmul(
                avE[:], V65[jt][:, p * VW : p * VW + P],
                expE[:, jt * NC : (jt + 1) * NC],
                start=(jt == 0), stop=(jt == KT - 1),
            )
            nc.tensor.matmul(
                avO[:], V65[jt][:, p * VW + 32 : p * VW + 32 + P],
                expO[:, jt * NC : (jt + 1) * NC],
                start=(jt == 0), stop=(jt == KT - 1),
            )
        # avE rows: 0-63 = OT_even, 64 = colsum_even
        # avO rows: 32 = colsum_odd, 64-127 = OT_odd
        if p == 0 and c == 0:
            dump_psum("avE", avE[:])
            dump_psum("avO", avO[:])
        rcf = sb.tile([P, NC], F32, tag="recipf")
        rc = sb.tile([P, NC], BF16, tag="recip")
        RECIP_FAST = False
        if RECIP_FAST:
            nc.vector.reciprocal_approx_fast(
                rcf[HD : HD + 1, :], avE[HD : HD + 1, :])
            nc.vector.reciprocal_approx_fast(
                rcf[32:33, :], avO[32:33, :])
        else:
            nc.vector.reciprocal(rcf[HD : HD + 1, :], avE[HD : HD + 1, :])
            nc.vector.reciprocal(rcf[32:33, :], avO[32:33, :])
        nc.vector.tensor_copy(rc[HD : HD + 1, :], rcf[HD : HD + 1, :])
        nc.vector.tensor_copy(rc[32:33, :], rcf[32:33, :])
        # broadcast each recip row to all 128 partitions via K=1 PE matmul
        rpsE = psB.tile([P, NC], F32, tag="ps", name="rpsE")
        rpsO = psB.tile([P, NC], F32, tag="ps", name="rpsO")
        nc.tensor.matmul(
            rpsE[:], ones128[HD : HD + 1, :], rc[HD : HD + 1, :],
            start=True, stop=True,
        )
        nc.tensor.matmul(
            rpsO[:], ones128[32:33, :], rc[32:33, :],
            start=True, stop=True,
        )
        Rt = sb.tile([P, NC], F32, tag="bcast")
        nc.vector.tensor_copy(Rt[0:HD, :], rpsE[0:HD, :])
        nc.vector.tensor_copy(Rt[HD:P, :], rpsO[HD:P, :])
        if p == 0 and c == 0:
            dump_sbuf("rc", rc[:])
            dump_sbuf("Rt", Rt[:])
        nc.vector.tensor_mul(
            OT[p][0:HD, c * NC : (c + 1) * NC], avE[0:HD, :], Rt[0:HD, :])
        nc.vector.tensor_mul(
            OT[p][HD:P, c * NC : (c + 1) * NC], avO[HD:P, :], Rt[HD:P, :])

    def emit_outproj(m, c):
        ps = psB.tile([P, NC], F32, tag="ps", name="ps")
        for kt in range(KT):
            nc.tensor.matmul(
                ps[:],
                OT[kt][:, m * P : (m + 1) * P],
                wo_t[kt][:, c * NC : (c + 1) * NC],
                start=(kt == 0), stop=False,
            )
        nc.tensor.matmul(
            ps[:], ones[0:1, :], bor[0:1, c * NC : (c + 1) * NC],
            start=False, stop=True,
        )
        osb = osbp.tile([P, NC], F32, tag="osb")
        nc.vector.tensor_copy(osb[:], ps[:])
        nc.sync.dma_start(
            io["out"][m * P : (m + 1) * P, c * NC : (c + 1) * NC], osb[:])

    # Attention blocks c-outer; once the c=0 half is fully reduced, the
    # output-projection chunks for s-tiles 0-3 interleave with the c=1
    # blocks to keep the PE dense (HAM warm).
    blocks = [(p, c) for c in range(NCH) for p in range(PAIRS)]
    op_chunks = [(m, cd) for m in range(MT) for cd in range(NCH)]
    pending = None
    emitted_op = 0
    for i, blk in enumerate(blocks):
        e = emit_scores(*blk)
        if pending is not None:
            emit_av(pending[0][0], pending[0][1], *pending[1])
        pending = (blk, e)
        if i >= 9:
            while emitted_op < min(2 * (i - 8), MT):
                emit_outproj(*op_chunks[emitted_op])
                emitted_op += 1
    emit_av(pending[0][0], pending[0][1], *pending[1])

    if dbg is not None:
        for m in range(MT):
            dump_sbuf("ot", OT[m][:], m * P)

    # ---------- remaining output projection ----------
    for m, cd in op_chunks[emitted_op:]:
        emit_outproj(m, cd)


def declare_io(nc):
    def din(name, shape, dt):
        return nc.dram_tensor(name, shape, dt, kind="ExternalInput").ap()

    io = {
        "xqT": din("xqT", [D, S], BF16),
        "xkT": din("xkT", [D, S], BF16),
        "xvT": din("xvT", [D, S], BF16),
        "wqT": din("wqT", [D, D], BF16),
        "wkT": din("wkT", [D, D], BF16),
        "wvT": din("wvT", [D, D], BF16),
        "woT": din("woT", [D, D], BF16),
        "bq_c": din("bq_c", [P, KT], F32),
        "bk_c": din("bk_c", [P, KT], F32),
        "bv_r": din("bv_r", [1, D], BF16),
        "bo_r": din("bo_r", [1, D], BF16),
        "out": nc.dram_tensor("out", [S, D], F32, kind="ExternalOutput").ap(),
    }
    return io


_NC_CACHE = {}


def get_nc():
    if "nc" not in _NC_CACHE:
        nc = bacc.Bacc(
            "TRN2",
            target_bir_lowering=False,
            debug=False,
            enable_asserts=False,
            num_devices=N_CORES,
        )
        io = declare_io(nc)
        with tile.TileContext(nc) as tc:
            with ExitStack() as ctx:
                build_body(ctx, tc, io)
        nc.compile()
        _NC_CACHE["nc"] = nc
    return _NC_CACHE["nc"]


def prep_inputs(query, key, value, Wq, bq, Wk, bk, Wv, bv, Wo, bo):
    bf = ml_dtypes.bfloat16
    f32 = np.float32

    def t16(a):
        return np.ascontiguousarray(np.asarray(a, dtype=f32).T).astype(bf)

    base = {
        "wqT": t16(Wq),
        "wkT": t16(Wk),
        "wvT": t16(Wv),
        "woT": t16(Wo),
        "bq_c": np.ascontiguousarray(
            np.asarray(bq, dtype=f32).reshape(KT, P).T),
        "bk_c": np.ascontiguousarray(
            np.asarray(bk, dtype=f32).reshape(KT, P).T),
        "bv_r": np.asarray(bv, dtype=f32).astype(bf).reshape(1, D),
        "bo_r": np.asarray(bo, dtype=f32).astype(bf).reshape(1, D),
    }
    in_maps = []
    for b in range(np.asarray(query).shape[0]):
        m = dict(base)
        m["xqT"] = t16(query[b])
        m["xkT"] = t16(key[b])
        m["xvT"] = t16(value[b])
        in_maps.append(m)
    return in_maps


def kernel(query, key, value, Wq, bq, Wk, bk, Wv, bv, Wo, bo, **run_kwargs):
    nc = get_nc()
    in_maps = prep_inputs(query, key, value, Wq, bq, Wk, bk, Wv, bv, Wo, bo)
    res = run_bass_kernel_spmd(
        nc, in_maps, core_ids=list(range(N_CORES)), **run_kwargs)
    out = np.stack(
        [res.results[b]["out"] for b in range(N_CORES)], axis=0
    ).astype(np.float32)
    if run_kwargs:
        kernel.last_results = res
    return out



# revision 14
# speedup vs baseline: 1.5299x; 1.5299x over previous
"""Multi-head attention Trainium2 kernel (v2).

B=8, S=1024, D=1024, H=16 heads, head_dim=64.
Sharding: pure data parallelism over batch — one batch element per
NeuronCore, weights replicated, no collectives.

Per-core dataflow (matmul operands bf16, fp32 PSUM accumulate):
  host:   xT = x.T (k-major) for q/k/v, WT = W.T for all weights, bf16.
  V[s,dv]  = sum_k xvT[k,s]*WvT[k,dv]  (+bv via DVE add w/ broadcast tile)
             scattered into V65 layout: per head pair
             [V_e(64) | ones(1) | zeros(31) | V_o(64)]  (160 wide)
  KT[do,s] = sum_k WkT[k,do]*xkT[k,s]  (+bk per-partition ACT-free DVE add)
  QT[do,s] likewise
  per (head-pair p, i-chunk c, j-block): scores as row-tiled concurrent
    pair (heads even on PE rows 0-63, odd on 64-127):
    sc[j,0:512]=E, sc[j,512:1024]=O; exp via ACT (scale=1/8) -> bf16 SBUF
  AV: av[:,0:512] accumulates [OT_e(64); Z_e@64; junk] over j-tiles,
      av[:,512:1024] accumulates [junk; Z_o@32; OT_o(64-127)]
  normalize: DVE recip_approx_fast on PSUM Z rows -> partition 0,
      gpsimd partition_broadcast, DVE mul -> OT[dv,s] bf16
  outT[do,s] = sum_dv WoT[dv,do].T @ OT[dv,s]  (+bo per-partition) -> bf16
  host transposes outT -> out.
"""

import numpy as np
import ml_dtypes
from contextlib import ExitStack

import concourse.bass as bass
import concourse.tile as tile
import concourse.mybir as mybir
from concourse import bacc
from concourse.bass_utils import run_bass_kernel_spmd

BF16 = mybir.dt.bfloat16
F32 = mybir.dt.float32
AF = mybir.ActivationFunctionType

S = 1024
D = 1024
H = 16
HD = 64
P = 128
KT = D // P      # 8 contraction tiles
MT = S // P      # 8 row tiles
NC = 512
NCH = S // NC    # 2 chunks
PAIRS = H // 2   # 8
VW = 160         # per-pair V65 width
N_CORES = 8


def build_body(ctx: ExitStack, tc, io, dbg=None):
    nc = tc.nc

    const = ctx.enter_context(tc.tile_pool(name="const", bufs=1))
    qkt = ctx.enter_context(tc.tile_pool(name="qkt", bufs=1))
    v65p = ctx.enter_context(tc.tile_pool(name="v65", bufs=1))
    otp = ctx.enter_context(tc.tile_pool(name="otp", bufs=1))
    xw = ctx.enter_context(tc.tile_pool(name="xw", bufs=2))
    expp = ctx.enter_context(tc.tile_pool(name="expp", bufs=2))
    sbp = ctx.enter_context(tc.tile_pool(name="sbp", bufs=2))
    osbp = ctx.enter_context(tc.tile_pool(name="osb", bufs=3))
    psp = ctx.enter_context(tc.tile_pool(name="psp", bufs=1, space="PSUM"))

    # ---- constants ----
    bqc = const.tile([P, KT], F32, tag="bqc", name="bqc")
    nc.sync.dma_start(bqc[:], io["bq_c"][:])
    bkc = const.tile([P, KT], F32, tag="bkc", name="bkc")
    nc.sync.dma_start(bkc[:], io["bk_c"][:])
    boc = const.tile([P, KT], F32, tag="boc", name="boc")
    nc.sync.dma_start(boc[:], io["bo_c"][:])
    bvr = const.tile([1, D], BF16, tag="bvr", name="bvr")
    nc.sync.dma_start(bvr[:], io["bv_r"][:])
    bvb = const.tile([P, D], BF16, tag="bvb", name="bvb")
    nc.gpsimd.partition_broadcast(bvb[:, :], bvr[0:1, :])
    onesb = const.tile([P, P], BF16, tag="onesb", name="onesb")
    nc.vector.memset(onesb[:], 1.0)

    # ---- persistent activation tiles ----
    QT = [qkt.tile([P, S], BF16, tag=f"qt{m}", name=f"qt{m}") for m in range(MT)]
    KTt = [qkt.tile([P, S], BF16, tag=f"kt{m}", name=f"ktt{m}") for m in range(MT)]
    V65 = [v65p.tile([P, PAIRS * VW], BF16, tag=f"v65_{m}", name=f"v65_{m}")
           for m in range(MT)]
    OT = [otp.tile([P, S], BF16, tag=f"ot{m}", name=f"ot{m}") for m in range(MT)]

    # ones column at col 64 of each 160-wide pair block; zero the gap
    for m in range(MT):
        v = V65[m].rearrange("p (pr w) -> p pr w", w=VW)
        nc.vector.memset(v[:, :, HD : HD + 1], 1.0)
        nc.vector.memset(v[:, :, HD + 1 : 96], 0.0)

    def load_xw(xdram, wdram):
        xt = [xw.tile([P, S], BF16, tag=f"x{k}", name=f"xt{k}") for k in range(KT)]
        wt = [xw.tile([P, D], BF16, tag=f"w{k}", name=f"wt{k}") for k in range(KT)]
        for k in range(KT):
            nc.sync.dma_start(xt[k][:], xdram[k * P : (k + 1) * P, :])
            nc.sync.dma_start(wt[k][:], wdram[k * P : (k + 1) * P, :])
        return xt, wt

    # ---------- V projection (V65 needed by all attention) ----------
    xt, wt = load_xw(io["xvT"], io["wvT"])
    bvb4 = bvb.rearrange("p (pr two x) -> p pr two x", two=2, x=HD)
    for m in range(MT):          # s tile
        ps = psp.tile([P, S], F32, tag="sc", bufs=2, name="psv")
        for k in range(KT):
            xs = xt[k][:, m * P : (m + 1) * P]
            nc.tensor.matmul(ps[:, 0:NC], xs, wt[k][:, 0:NC],
                             start=(k == 0), stop=(k == KT - 1))
            nc.tensor.matmul(ps[:, NC:S], xs, wt[k][:, NC:S],
                             start=(k == 0), stop=(k == KT - 1))
        v = V65[m].rearrange("p (pr w) -> p pr w", w=VW)
        for c in range(NCH):
            psv = ps[:, c * NC : (c + 1) * NC].rearrange(
                "p (pr two x) -> p pr two x", two=2, x=HD)
            pr0 = c * 4
            nc.vector.tensor_add(
                v[:, pr0 : pr0 + 4, 0:HD], psv[:, :, 0, :],
                bvb4[:, pr0 : pr0 + 4, 0, :])
            nc.vector.tensor_add(
                v[:, pr0 : pr0 + 4, 96:VW], psv[:, :, 1, :],
                bvb4[:, pr0 : pr0 + 4, 1, :])

    # ---------- K projection ----------
    xt, wt = load_xw(io["xkT"], io["wkT"])
    for m in range(MT):          # do tile
        ps = psp.tile([P, S], F32, tag="sc", bufs=2, name="psk")
        for k in range(KT):
            ws = wt[k][:, m * P : (m + 1) * P]
            nc.tensor.matmul(ps[:, 0:NC], ws, xt[k][:, 0:NC],
                             start=(k == 0), stop=(k == KT - 1))
            nc.tensor.matmul(ps[:, NC:S], ws, xt[k][:, NC:S],
                             start=(k == 0), stop=(k == KT - 1))
        nc.vector.tensor_scalar_add(KTt[m][:], ps[:], bkc[:, m : m + 1])

    # ---------- Q projection ----------
    xt, wt = load_xw(io["xqT"], io["wqT"])
    for m in range(MT):
        ps = psp.tile([P, S], F32, tag="sc", bufs=2, name="psq")
        for k in range(KT):
            ws = wt[k][:, m * P : (m + 1) * P]
            nc.tensor.matmul(ps[:, 0:NC], ws, xt[k][:, 0:NC],
                             start=(k == 0), stop=(k == KT - 1))
            nc.tensor.matmul(ps[:, NC:S], ws, xt[k][:, NC:S],
                             start=(k == 0), stop=(k == KT - 1))
        nc.vector.tensor_scalar_add(QT[m][:], ps[:], bqc[:, m : m + 1])

    # WoT loads overlap attention (w slots rotate after Q projection)
    wo_t = [xw.tile([P, D], BF16, tag=f"w{k}", name=f"wo{k}") for k in range(KT)]
    for k in range(KT):
        nc.sync.dma_start(wo_t[k][:], io["woT"][k * P : (k + 1) * P, :])

    # ---------- attention ----------
    def emit_scores(p, c):
        """exp(scores/8) for heads 2p (E) and 2p+1 (O), i-chunk c.

        Layout: expEO[:, j*1024+0:512] = E, [j*1024+512:1024] = O.
        E/O matmuls are row-tiled (rows 0-63 / 64-127) -> run concurrently.
        """
        expEO = expp.tile([P, KT * S], BF16, tag="expEO", name="expEO")
        qE = QT[p][0:HD, c * NC : (c + 1) * NC]
        qO = QT[p][HD:P, c * NC : (c + 1) * NC]
        for j in range(MT):
            sc = psp.tile([P, S], F32, tag="sc", bufs=2, name="sc")
            nc.tensor.matmul(sc[:, 0:NC], KTt[p][0:HD, j * P : (j + 1) * P],
                             qE, start=True, stop=True)
            nc.tensor.matmul(sc[:, NC:S], KTt[p][HD:P, j * P : (j + 1) * P],
                             qO, start=True, stop=True)
            nc.scalar.activation(
                expEO[:, j * S : (j + 1) * S], sc[:], AF.Exp, scale=0.125)
        return expEO

    def emit_av(p, c, expEO):
        av = psp.tile([P, S], F32, tag="av", bufs=1, name="av")
        for jt in range(KT):
            nc.tensor.matmul(
                av[:, 0:NC], V65[jt][:, p * VW : p * VW + P],
                expEO[:, jt * S : jt * S + NC],
                start=(jt == 0), stop=(jt == KT - 1))
            nc.tensor.matmul(
                av[:, NC:S], V65[jt][:, p * VW + 32 : p * VW + VW],
                expEO[:, jt * S + NC : (jt + 1) * S],
                start=(jt == 0), stop=(jt == KT - 1))
        return av

    def emit_norm(p, c, av):
        # av rows: [0:64,0:512]=OT_e, [64,0:512]=Z_e,
        #          [32,512:1024]=Z_o, [64:128,512:1024]=OT_o
        rc = sbp.tile([P, S], BF16, tag="rc", name="rc")
        nc.vector.tensor_copy(rc[HD : HD + 1, 0:NC], av[HD : HD + 1, 0:NC])
        nc.vector.tensor_copy(rc[32:33, NC:S], av[32:33, NC:S])
        # broadcast raw Z to all partitions: concurrent K=1 matmul pair
        # (E: row strip 2 / col strips 0-1; O: row strip 1 / col strips 2-3)
        rps = psp.tile([P, NC], F32, tag="aux", bufs=2, name="rps")
        nc.tensor.matmul(rps[0:HD, :], onesb[HD : HD + 1, 0:HD],
                         rc[HD : HD + 1, 0:NC], start=True, stop=True)
        nc.tensor.matmul(rps[HD:P, :], onesb[32:33, 0:HD],
                         rc[32:33, NC:S], start=True, stop=True)
        # full-tile base-0 approx reciprocal (custom-DVE op is only
        # correct at base partition 0)
        rbcf = sbp.tile([P, NC], F32, tag="rbcf", name="rbcf")
        nc.vector.reciprocal_approx_fast(rbcf[:], rps[:])
        if dbg is not None and (p, c) == (0, 0):
            build_body.dump("rbc0", rbcf[:])
        nc.vector.tensor_mul(
            OT[p][0:HD, c * NC : (c + 1) * NC], av[0:HD, 0:NC], rbcf[0:HD, :])
        nc.vector.tensor_mul(
            OT[p][HD:P, c * NC : (c + 1) * NC], av[HD:P, NC:S], rbcf[HD:P, :])

    def emit_outproj(m, cd):
        """outT[m-do-block, cd-s-chunk]: stationary WoT slices."""
        ps = psp.tile([P, NC], F32, tag="aux", bufs=2, name="psop")
        for kt in range(KT):
            nc.tensor.matmul(
                ps[:], wo_t[kt][:, m * P : (m + 1) * P],
                OT[kt][:, cd * NC : (cd + 1) * NC],
                start=(kt == 0), stop=(kt == KT - 1))
        osb = osbp.tile([P, NC], BF16, tag="osb", name="osb")
        nc.vector.tensor_scalar_add(osb[:], ps[:], boc[:, m : m + 1])
        nc.sync.dma_start(
            io["outT"][m * P : (m + 1) * P, cd * NC : (cd + 1) * NC], osb[:])

    def dump(key, ap, psum=False):
        if dbg is not None and key in dbg:
            if psum:
                t = osbp.tile(list(ap.shape), F32, tag=f"dbg_{key}", bufs=1,
                              name="dbgcp")
                nc.vector.tensor_copy(t[:], ap)
                ap = t[:]
            nc.sync.dma_start(dbg[key][:, :], ap)
    build_body.dump = dump

    blocks = [(p, c) for c in range(NCH) for p in range(PAIRS)]
    pending = None
    emitted_op = 0
    for i, blk in enumerate(blocks):
        e = emit_scores(*blk)
        if i == 0:
            dump("exp0", e[:])
        if pending is not None:
            av = emit_av(pending[0][0], pending[0][1], pending[1])
            if i == 1:
                dump("av0", av[:], psum=True)
            emit_norm(pending[0][0], pending[0][1], av)
        pending = (blk, e)
        # after the c=0 half is fully reduced (blocks 0-7 normalized by
        # end of i=8), interleave cd=0 out-projection chunks
        if i >= 9:
            emit_outproj(i - 9, 0)
            emitted_op += 1
    av = emit_av(pending[0][0], pending[0][1], pending[1])
    emit_norm(pending[0][0], pending[0][1], av)

    # ---------- remaining output projection ----------
    for m in range(emitted_op, MT):
        emit_outproj(m, 0)
    for m in range(MT):
        emit_outproj(m, 1)


def declare_io(nc):
    def din(name, shape, dt):
        return nc.dram_tensor(name, shape, dt, kind="ExternalInput").ap()

    io = {
        "xqT": din("xqT", [D, S], BF16),
        "xkT": din("xkT", [D, S], BF16),
        "xvT": din("xvT", [D, S], BF16),
        "wqT": din("wqT", [D, D], BF16),
        "wkT": din("wkT", [D, D], BF16),
        "wvT": din("wvT", [D, D], BF16),
        "woT": din("woT", [D, D], BF16),
        "bq_c": din("bq_c", [P, KT], F32),
        "bk_c": din("bk_c", [P, KT], F32),
        "bo_c": din("bo_c", [P, KT], F32),
        "bv_r": din("bv_r", [1, D], BF16),
        "outT": nc.dram_tensor("outT", [D, S], BF16, kind="ExternalOutput").ap(),
    }
    return io


_NC_CACHE = {}


def get_nc():
    if "nc" not in _NC_CACHE:
        nc = bacc.Bacc(
            "TRN2",
            target_bir_lowering=False,
            debug=False,
            enable_asserts=False,
            num_devices=N_CORES,
        )
        io = declare_io(nc)
        with tile.TileContext(nc) as tc:
            with ExitStack() as ctx:
                build_body(ctx, tc, io)
        nc.compile()
        _NC_CACHE["nc"] = nc
    return _NC_CACHE["nc"]


def prep_inputs(query, key, value, Wq, bq, Wk, bk, Wv, bv, Wo, bo):
    bf = ml_dtypes.bfloat16
    f32 = np.float32

    def t16(a):
        return np.ascontiguousarray(np.asarray(a, dtype=f32).T).astype(bf)

    base = {
        "wqT": t16(Wq),
        "wkT": t16(Wk),
        "wvT": t16(Wv),
        "woT": t16(Wo),
        "bq_c": np.ascontiguousarray(
            np.asarray(bq, dtype=f32).reshape(KT, P).T),
        "bk_c": np.ascontiguousarray(
            np.asarray(bk, dtype=f32).reshape(KT, P).T),
        "bo_c": np.ascontiguousarray(
            np.asarray(bo, dtype=f32).reshape(KT, P).T),
        "bv_r": np.asarray(bv, dtype=f32).astype(bf).reshape(1, D),
    }
    in_maps = []
    for b in range(np.asarray(query).shape[0]):
        m = dict(base)
        m["xqT"] = t16(query[b])
        m["xkT"] = t16(key[b])
        m["xvT"] = t16(value[b])
        in_maps.append(m)
    return in_maps


def kernel(query, key, value, Wq, bq, Wk, bk, Wv, bv, Wo, bo, **run_kwargs):
    nc = get_nc()
    in_maps = prep_inputs(query, key, value, Wq, bq, Wk, bk, Wv, bv, Wo, bo)
    res = run_bass_kernel_spmd(
        nc, in_maps, core_ids=list(range(N_CORES)), **run_kwargs)
    out = np.stack(
        [res.results[b]["outT"].astype(np.float32).T for b in range(N_CORES)],
        axis=0)
    if run_kwargs:
        kernel.last_results = res
    return out


# revision 17
# speedup vs baseline: 1.6435x; 1.0742x over previous
"""Multi-head attention Trainium2 kernel (v2).

B=8, S=1024, D=1024, H=16 heads, head_dim=64.
Sharding: pure data parallelism over batch — one batch element per
NeuronCore, weights replicated, no collectives.

Per-core dataflow (matmul operands bf16, fp32 PSUM accumulate):
  host:   xT = x.T (k-major) for q/k/v, WT = W.T for all weights, bf16.
  V[s,dv]  = sum_k xvT[k,s]*WvT[k,dv]  (+bv via DVE add w/ broadcast tile)
             scattered into V65 layout: per head pair
             [V_e(64) | ones(1) | zeros(31) | V_o(64)]  (160 wide)
  KT[do,s] = sum_k WkT[k,do]*xkT[k,s]  (+bk per-partition ACT-free DVE add)
  QT[do,s] likewise
  per (head-pair p, i-chunk c, j-block): scores as row-tiled concurrent
    pair (heads even on PE rows 0-63, odd on 64-127):
    sc[j,0:512]=E, sc[j,512:1024]=O; exp via ACT (scale=1/8) -> bf16 SBUF
  AV: av[:,0:512] accumulates [OT_e(64); Z_e@64; junk] over j-tiles,
      av[:,512:1024] accumulates [junk; Z_o@32; OT_o(64-127)]
  normalize: DVE recip_approx_fast on PSUM Z rows -> partition 0,
      gpsimd partition_broadcast, DVE mul -> OT[dv,s] bf16
  outT[do,s] = sum_dv WoT[dv,do].T @ OT[dv,s]  (+bo per-partition) -> bf16
  host transposes outT -> out.
"""

import numpy as np
import ml_dtypes
from contextlib import ExitStack

import concourse.bass as bass
import concourse.tile as tile
import concourse.mybir as mybir
from concourse import bacc
from concourse.bass_utils import run_bass_kernel_spmd

BF16 = mybir.dt.bfloat16
F32 = mybir.dt.float32
AF = mybir.ActivationFunctionType

S = 1024
D = 1024
H = 16
HD = 64
P = 128
KT = D // P      # 8 contraction tiles
MT = S // P      # 8 row tiles
NC = 512
NCH = S // NC    # 2 chunks
PAIRS = H // 2   # 8
VW = 160         # per-pair V65 width
N_CORES = 8


def build_body(ctx: ExitStack, tc, io, dbg=None):
    nc = tc.nc

    const = ctx.enter_context(tc.tile_pool(name="const", bufs=1))
    qkt = ctx.enter_context(tc.tile_pool(name="qkt", bufs=1))
    v65p = ctx.enter_context(tc.tile_pool(name="v65", bufs=1))
    otp = ctx.enter_context(tc.tile_pool(name="otp", bufs=1))
    xw = ctx.enter_context(tc.tile_pool(name="xw", bufs=2))
    wop = ctx.enter_context(tc.tile_pool(name="wop", bufs=1))
    expp = ctx.enter_context(tc.tile_pool(name="expp", bufs=2))
    sbp = ctx.enter_context(tc.tile_pool(name="sbp", bufs=2))
    osbp = ctx.enter_context(tc.tile_pool(name="osb", bufs=3))
    psp = ctx.enter_context(tc.tile_pool(name="psp", bufs=1, space="PSUM"))

    # ---- constants ----
    bqc = const.tile([P, KT], F32, tag="bqc", name="bqc")
    nc.sync.dma_start(bqc[:], io["bq_c"][:])
    bkc = const.tile([P, KT], F32, tag="bkc", name="bkc")
    nc.sync.dma_start(bkc[:], io["bk_c"][:])
    boc = const.tile([P, KT], F32, tag="boc", name="boc")
    nc.sync.dma_start(boc[:], io["bo_c"][:])
    bvr = const.tile([1, D], BF16, tag="bvr", name="bvr")
    nc.sync.dma_start(bvr[:], io["bv_r"][:])
    bvb = const.tile([P, D], BF16, tag="bvb", name="bvb")
    nc.gpsimd.partition_broadcast(bvb[:, :], bvr[0:1, :])
    onesb = const.tile([P, P], BF16, tag="onesb", name="onesb")
    nc.vector.memset(onesb[:], 1.0)

    # ---- persistent activation tiles ----
    QT = [qkt.tile([P, S], BF16, tag=f"qt{m}", name=f"qt{m}") for m in range(MT)]
    KTt = [qkt.tile([P, S], BF16, tag=f"kt{m}", name=f"ktt{m}") for m in range(MT)]
    V65 = [v65p.tile([P, PAIRS * VW], BF16, tag=f"v65_{m}", name=f"v65_{m}")
           for m in range(MT)]
    OT = [otp.tile([P, S], BF16, tag=f"ot{m}", name=f"ot{m}") for m in range(MT)]

    # ones column at col 64 of each 160-wide pair block; zero the gap
    for m in range(MT):
        v = V65[m].rearrange("p (pr w) -> p pr w", w=VW)
        nc.vector.memset(v[:, :, HD : HD + 1], 1.0)
        nc.vector.memset(v[:, :, HD + 1 : 96], 0.0)

    def load_xw(xdram, wdram):
        xt = [xw.tile([P, S], BF16, tag=f"x{k}", name=f"xt{k}") for k in range(KT)]
        wt = [xw.tile([P, D], BF16, tag=f"w{k}", name=f"wt{k}") for k in range(KT)]
        for k in range(KT):
            nc.sync.dma_start(xt[k][:], xdram[k * P : (k + 1) * P, :])
            nc.sync.dma_start(wt[k][:], wdram[k * P : (k + 1) * P, :])
        return xt, wt

    # ---------- V projection (V65 needed by all attention) ----------
    xt, wt = load_xw(io["xvT"], io["wvT"])
    bvb4 = bvb.rearrange("p (pr two x) -> p pr two x", two=2, x=HD)
    for m in range(MT):          # s tile
        ps = psp.tile([P, S], F32, tag="sc", bufs=2, name="psv")
        for k in range(KT):
            xs = xt[k][:, m * P : (m + 1) * P]
            nc.tensor.matmul(ps[:, 0:NC], xs, wt[k][:, 0:NC],
                             start=(k == 0), stop=(k == KT - 1))
            nc.tensor.matmul(ps[:, NC:S], xs, wt[k][:, NC:S],
                             start=(k == 0), stop=(k == KT - 1))
        v = V65[m].rearrange("p (pr w) -> p pr w", w=VW)
        for c in range(NCH):
            psv = ps[:, c * NC : (c + 1) * NC].rearrange(
                "p (pr two x) -> p pr two x", two=2, x=HD)
            pr0 = c * 4
            nc.vector.tensor_add(
                v[:, pr0 : pr0 + 4, 0:HD], psv[:, :, 0, :],
                bvb4[:, pr0 : pr0 + 4, 0, :])
            nc.vector.tensor_add(
                v[:, pr0 : pr0 + 4, 96:VW], psv[:, :, 1, :],
                bvb4[:, pr0 : pr0 + 4, 1, :])

    # ---------- K/Q projections ----------
    # m=0 is emitted up-front (full width, "sc" psum); m>=1 chunks are
    # interleaved into the attention blocks via the filler queue below
    # ("aux" psum, one 512-wide half at a time).
    xtk, wtk = load_xw(io["xkT"], io["wkT"])
    xtq, wtq = load_xw(io["xqT"], io["wqT"])

    def kq_full(wt, xt, dest, bias, m):
        ps = psp.tile([P, S], F32, tag="sc", bufs=2, name="pskq")
        for k in range(KT):
            ws = wt[k][:, m * P : (m + 1) * P]
            nc.tensor.matmul(ps[:, 0:NC], ws, xt[k][:, 0:NC],
                             start=(k == 0), stop=(k == KT - 1))
            nc.tensor.matmul(ps[:, NC:S], ws, xt[k][:, NC:S],
                             start=(k == 0), stop=(k == KT - 1))
        nc.vector.tensor_scalar_add(dest[m][:], ps[:], bias[:, m : m + 1])

    def kq_half(wt, xt, dest, bias, m, half):
        ps = psp.tile([P, NC], F32, tag="aux", bufs=2, name="pskq2")
        lo = half * NC
        for k in range(KT):
            nc.tensor.matmul(ps[:], wt[k][:, m * P : (m + 1) * P],
                             xt[k][:, lo : lo + NC],
                             start=(k == 0), stop=(k == KT - 1))
        nc.vector.tensor_scalar_add(
            dest[m][:, lo : lo + NC], ps[:], bias[:, m : m + 1])

    kq_full(wtk, xtk, KTt, bkc, 0)
    kq_full(wtq, xtq, QT, bqc, 0)

    # WoT loads overlap attention
    wo_t = [wop.tile([P, D], BF16, tag=f"wo{k}", name=f"wo{k}")
            for k in range(KT)]
    for k in range(KT):
        nc.sync.dma_start(wo_t[k][:], io["woT"][k * P : (k + 1) * P, :])

    # ---------- attention ----------
    def emit_scores(p, c):
        """exp(scores/8) for heads 2p (E) and 2p+1 (O), i-chunk c.

        Layout: expEO[:, j*1024+0:512] = E, [j*1024+512:1024] = O.
        E/O matmuls are row-tiled (rows 0-63 / 64-127) -> run concurrently.
        """
        expEO = expp.tile([P, KT * S], BF16, tag="expEO", name="expEO")
        qE = QT[p][0:HD, c * NC : (c + 1) * NC]
        qO = QT[p][HD:P, c * NC : (c + 1) * NC]
        for j in range(MT):
            sc = psp.tile([P, S], F32, tag="sc", bufs=2, name="sc")
            nc.tensor.matmul(sc[:, 0:NC], KTt[p][0:HD, j * P : (j + 1) * P],
                             qE, start=True, stop=True)
            nc.tensor.matmul(sc[:, NC:S], KTt[p][HD:P, j * P : (j + 1) * P],
                             qO, start=True, stop=True)
            nc.scalar.activation(
                expEO[:, j * S : (j + 1) * S], sc[:], AF.Exp, scale=0.125)
        return expEO

    def emit_av(p, c, expEO):
        av = psp.tile([P, S], F32, tag="av", bufs=1, name="av")
        for jt in range(KT):
            nc.tensor.matmul(
                av[:, 0:NC], V65[jt][:, p * VW : p * VW + P],
                expEO[:, jt * S : jt * S + NC],
                start=(jt == 0), stop=(jt == KT - 1))
            nc.tensor.matmul(
                av[:, NC:S], V65[jt][:, p * VW + 32 : p * VW + VW],
                expEO[:, jt * S + NC : (jt + 1) * S],
                start=(jt == 0), stop=(jt == KT - 1))
        return av

    def emit_norm(p, c, av):
        # av rows: [0:64,0:512]=OT_e, [64,0:512]=Z_e,
        #          [32,512:1024]=Z_o, [64:128,512:1024]=OT_o
        rc = sbp.tile([P, S], BF16, tag="rc", name="rc")
        nc.vector.tensor_copy(rc[HD : HD + 1, 0:NC], av[HD : HD + 1, 0:NC])
        nc.vector.tensor_copy(rc[32:33, NC:S], av[32:33, NC:S])
        # broadcast raw Z to all partitions: concurrent K=1 matmul pair
        # (E: row strip 2 / col strips 0-1; O: row strip 1 / col strips 2-3)
        rps = psp.tile([P, NC], F32, tag="aux", bufs=2, name="rps")
        nc.tensor.matmul(rps[0:HD, :], onesb[HD : HD + 1, 0:HD],
                         rc[HD : HD + 1, 0:NC], start=True, stop=True)
        nc.tensor.matmul(rps[HD:P, :], onesb[32:33, 0:HD],
                         rc[32:33, NC:S], start=True, stop=True)
        # full-tile base-0 approx reciprocal (custom-DVE op is only
        # correct at base partition 0)
        rbcf = sbp.tile([P, NC], F32, tag="rbcf", name="rbcf")
        nc.vector.reciprocal_approx_fast(rbcf[:], rps[:])
        if dbg is not None and (p, c) == (0, 0):
            build_body.dump("rbc0", rbcf[:])
        nc.vector.tensor_mul(
            OT[p][0:HD, c * NC : (c + 1) * NC], av[0:HD, 0:NC], rbcf[0:HD, :])
        nc.vector.tensor_mul(
            OT[p][HD:P, c * NC : (c + 1) * NC], av[HD:P, NC:S], rbcf[HD:P, :])

    def emit_outproj(m, cd):
        """outT[m-do-block, cd-s-chunk]: stationary WoT slices."""
        ps = psp.tile([P, NC], F32, tag="aux", bufs=2, name="psop")
        for kt in range(KT):
            nc.tensor.matmul(
                ps[:], wo_t[kt][:, m * P : (m + 1) * P],
                OT[kt][:, cd * NC : (cd + 1) * NC],
                start=(kt == 0), stop=(kt == KT - 1))
        osb = osbp.tile([P, NC], BF16, tag="osb", name="osb")
        nc.vector.tensor_scalar_add(osb[:], ps[:], boc[:, m : m + 1])
        nc.sync.dma_start(
            io["outT"][m * P : (m + 1) * P, cd * NC : (cd + 1) * NC], osb[:])

    def dump(key, ap, psum=False):
        if dbg is not None and key in dbg:
            if psum:
                t = osbp.tile(list(ap.shape), F32, tag=f"dbg_{key}", bufs=1,
                              name="dbgcp")
                nc.vector.tensor_copy(t[:], ap)
                ap = t[:]
            nc.sync.dma_start(dbg[key][:, :], ap)
    build_body.dump = dump

    # ---------- block scheduler ----------
    # The PE engine queue is in-order: a score matmul stalled on the
    # exp-pace (sc bufs=2) blocks everything behind it.  So filler work
    # (AV of the previous block, K/Q projection chunks, out-projection
    # chunks, normalize broadcasts) is emitted BEFORE each score pair,
    # in small quanta drained from a FIFO.
    from collections import deque
    filler = deque()

    def drain(n):
        for _ in range(n):
            if not filler:
                return
            filler.popleft()()

    def queue_av_norm(p, c, expEO):
        av_ref = [None]

        def mk_av(jt):
            def f():
                if av_ref[0] is None:
                    av_ref[0] = psp.tile([P, S], F32, tag="av", bufs=1,
                                         name="av")
                av = av_ref[0]
                nc.tensor.matmul(
                    av[:, 0:NC], V65[jt][:, p * VW : p * VW + P],
                    expEO[:, jt * S : jt * S + NC],
                    start=(jt == 0), stop=(jt == KT - 1))
                nc.tensor.matmul(
                    av[:, NC:S], V65[jt][:, p * VW + 32 : p * VW + VW],
                    expEO[:, jt * S + NC : (jt + 1) * S],
                    start=(jt == 0), stop=(jt == KT - 1))
            return f
        for jt in range(KT):
            filler.append(mk_av(jt))

        def f_norm():
            if dbg is not None and (p, c) == (0, 0):
                dump("av0", av_ref[0][:], psum=True)
            emit_norm(p, c, av_ref[0])
        filler.append(f_norm)

    blocks = [(p, c) for c in range(NCH) for p in range(PAIRS)]
    pending = None
    op_next = 0
    for i, blk in enumerate(blocks):
        p, c = blk
        # queue this block's filler work (runs during ACT-bound scores)
        if c == 0 and 1 <= p + 1 < MT:
            mnext = p + 1
            for half in range(NCH):
                filler.append(
                    lambda m=mnext, h=half: kq_half(wtk, xtk, KTt, bkc, m, h))
            for half in range(NCH):
                filler.append(
                    lambda m=mnext, h=half: kq_half(wtq, xtq, QT, bqc, m, h))
        if i >= 9:
            for _ in range(2):
                if op_next < MT:
                    filler.append(lambda m=op_next: emit_outproj(m, 0))
                    op_next += 1
        if pending is not None:
            queue_av_norm(pending[0][0], pending[0][1], pending[1])

        # scores for this block, draining filler between j-steps
        expEO = expp.tile([P, KT * S], BF16, tag="expEO", name="expEO")
        qE = QT[p][0:HD, c * NC : (c + 1) * NC]
        qO = QT[p][HD:P, c * NC : (c + 1) * NC]
        for j in range(MT):
            drain(2)
            sc = psp.tile([P, S], F32, tag="sc", bufs=2, name="sc")
            nc.tensor.matmul(sc[:, 0:NC], KTt[p][0:HD, j * P : (j + 1) * P],
                             qE, start=True, stop=True)
            nc.tensor.matmul(sc[:, NC:S], KTt[p][HD:P, j * P : (j + 1) * P],
                             qO, start=True, stop=True)
            nc.scalar.activation(
                expEO[:, j * S : (j + 1) * S], sc[:], AF.Exp, scale=0.125)
        if i == 0:
            dump("exp0", expEO[:])
        pending = (blk, expEO)

    drain(len(filler))
    queue_av_norm(pending[0][0], pending[0][1], pending[1])
    drain(len(filler))

    # ---------- remaining output projection ----------
    for m in range(op_next, MT):
        emit_outproj(m, 0)
    for m in range(MT):
        emit_outproj(m, 1)


def declare_io(nc):
    def din(name, shape, dt):
        return nc.dram_tensor(name, shape, dt, kind="ExternalInput").ap()

    io = {
        "xqT": din("xqT", [D, S], BF16),
        "xkT": din("xkT", [D, S], BF16),
        "xvT": din("xvT", [D, S], BF16),
        "wqT": din("wqT", [D, D], BF16),
        "wkT": din("wkT", [D, D], BF16),
        "wvT": din("wvT", [D, D], BF16),
        "woT": din("woT", [D, D], BF16),
        "bq_c": din("bq_c", [P, KT], F32),
        "bk_c": din("bk_c", [P, KT], F32),
        "bo_c": din("bo_c", [P, KT], F32),
        "bv_r": din("bv_r", [1, D], BF16),
        "outT": nc.dram_tensor("outT", [D, S], BF16, kind="ExternalOutput").ap(),
    }
    return io


_NC_CACHE = {}


def get_nc():
    if "nc" not in _NC_CACHE:
        nc = bacc.Bacc(
            "TRN2",
            target_bir_lowering=False,
            debug=False,
            enable_asserts=False,
            num_devices=N_CORES,
        )
        io = declare_io(nc)
        with tile.TileContext(nc) as tc:
            with ExitStack() as ctx:
                build_body(ctx, tc, io)
        nc.compile()
        _NC_CACHE["nc"] = nc
    return _NC_CACHE["nc"]


def prep_inputs(query, key, value, Wq, bq, Wk, bk, Wv, bv, Wo, bo):
    bf = ml_dtypes.bfloat16
    f32 = np.float32

    def t16(a):
        return np.ascontiguousarray(np.asarray(a, dtype=f32).T).astype(bf)

    base = {
        "wqT": t16(Wq),
        "wkT": t16(Wk),
        "wvT": t16(Wv),
        "woT": t16(Wo),
        "bq_c": np.ascontiguousarray(
            np.asarray(bq, dtype=f32).reshape(KT, P).T),
        "bk_c": np.ascontiguousarray(
            np.asarray(bk, dtype=f32).reshape(KT, P).T),
        "bo_c": np.ascontiguousarray(
            np.asarray(bo, dtype=f32).reshape(KT, P).T),
        "bv_r": np.asarray(bv, dtype=f32).astype(bf).reshape(1, D),
    }
    in_maps = []
    for b in range(np.asarray(query).shape[0]):
        m = dict(base)
        m["xqT"] = t16(query[b])
        m["xkT"] = t16(key[b])
        m["xvT"] = t16(value[b])
        in_maps.append(m)
    return in_maps


def kernel(query, key, value, Wq, bq, Wk, bk, Wv, bv, Wo, bo, **run_kwargs):
    nc = get_nc()
    in_maps = prep_inputs(query, key, value, Wq, bq, Wk, bk, Wv, bv, Wo, bo)
    res = run_bass_kernel_spmd(
        nc, in_maps, core_ids=list(range(N_CORES)), **run_kwargs)
    out = np.stack(
        [res.results[b]["outT"].astype(np.float32).T for b in range(N_CORES)],
        axis=0)
    if run_kwargs:
        kernel.last_results = res
    return out
